# revision 4
# baseline (speedup 1.0000x reference)
"""MoE (top-2 of 8 experts, d=1024) — mixed bf16/fp8 hidden-split Bass kernel
for 8 trn2 cores.

Two stacked ideas:

1. Hidden-split expert parallelism (as before): each expert's MLP is split
   in half along the HIDDEN dimension (512 units each); experts sorted by
   routed-token count and paired big-with-small onto core pairs; core 2p
   takes hidden-half 0 of pair p's two experts, core 2p+1 takes half 1.
   Partial y outputs combine on the host along with the top-2 gate weights.

2. Score-weighted mixed precision: the combine weight of a routed
   (token, expert) pair is its RAW gate score, so pairs with small |score|
   contribute proportionally small absolute error to the output. Each
   expert's routed set is split into a bf16 class (the Qb highest-|score|
   pairs) and an fp8 class (the rest). fp8 pairs run both MLP layers in
   fp8e4m3 with MatmulPerfMode.DoubleRow, which processes a 256-deep
   contraction per pass — measured ~2.13x the sustained bf16 row rate on
   this silicon (the PE's sustained-power row rate, ~2.03 Grows/s bf16, is
   the binding roofline; LDWEIGHTS and PSUM bank patterns measure free).
   Device-accurate numpy emulation puts the end-to-end max-rel error at
   ~1.2e-2 for Qb=1024 (50% fp8) vs the 2e-2 gate.

   fp8 scaling: W1, W2 are quantized as fp8(16*W); biases pre-scaled on the
   host (16*b1, 256*b2); h is stored as fp8(16*h_true) straight out of the
   bias+relu op, and the final partial y (= 256*y_true) is stored bf16 and
   divided by 256 during the host combine. No extra device ops vs bf16.

Per-core program (SPMD, identical): four sections Ab(Qb) Af(QfA) Bb(Qb)
Bf(QfB), where Qb is a uniform bf16 capacity (zero bf16 padding) and the
per-expert count variance lives in the half-cost fp8 sections.
Software-pipelined chunks across sections as before: chunk s's layer-1
matmuls interleave with chunk s-1's layer-2 matmuls; tail chunks first;
all DRAM packed 128-partition-major.
"""

import numpy as np
import ml_dtypes

import concourse.bass as bass
import concourse.mybir as mybir
import concourse.tile as tile
from concourse import bacc
from concourse.bass_utils import run_bass_kernel_spmd

# Problem shapes (hardcoded per contract)
D = 1024   # d_model == d_hidden
HD = 512   # hidden half per shard
N_EXPERTS = 8
TOP_K = 2
N_CORES = 8
B, T = 4, 2048
N_TOKENS = B * T

F32 = mybir.dt.float32
BF16 = mybir.dt.bfloat16
F8 = mybir.dt.float8e4
BF = ml_dtypes.bfloat16
F8NP = ml_dtypes.float8_e4m3
KC = D // 128    # layer-1 contraction chunks (8)
KH = HD // 128   # layer-2 contraction chunks (4) == layer-1 output chunks
MC = D // 128    # layer-2 output chunks (8)
NT = 512         # tokens per matmul (moving free dim; one PSUM bank fp32)
CGRAIN = 32      # capacity granularity

QB = 1024        # bf16 pairs per expert (rest go fp8); tune vs error budget
WSCALE = 16.0    # fp8 weight scale; y partials come out scaled by WSCALE^2
DR = mybir.MatmulPerfMode.DoubleRow


def chunk_list(C):
    """Chunk sizes in processing order: tail (if any) first, then 512s."""
    assert C % CGRAIN == 0 and C > 0
    if C < NT:
        return [C]
    sizes = [NT] * (C // NT)
    if C % NT:
        sizes = [C % NT] + sizes
    return sizes


def build_moe_expert_kernel(C, repeat: int = 1, split_w: int = 2,
                            wdouble: bool = True, unroll: int = 1,
                            hw_loop: bool = True) -> bacc.Bacc:
    """C = (Qb, QfA, QfB). Sections: Ab(Qb, bf16), Af(QfA, fp8),
    Bb(Qb, bf16), Bf(QfB, fp8).

    DRAM inputs (packed partition-major):
      x{g}  [nfull, 128, KC, NT] (+ x{g}t [128, KC, tail])   bf16 | fp8
      w1{g} [128, KC, HD], w2{g} [128, KH, D]                bf16 | fp8
      b1{g} [128, KH], b2{g} [128, MC]                       f32 (prescaled
                                                             for fp8)
    Outputs: y{g} [nfull, 128, MC, NT] (+ y{g}t) bf16 partial sums
    (fp8 sections' y is 256x the true partial; host divides).
    `repeat` wraps the body in a hardware loop for slope timing; `wdouble`
    double-buffers bf16 weights and unrolls 2 iterations per trip so
    iteration k+1's weight reloads overlap iteration k's compute. fp8
    weights/biases are single-buffered (their reload window is wide).
    """
    Qb, QfA, QfB = C
    if wdouble:
        unroll = 2
        repeat = max(1, (repeat + 1) // 2)
    nc = bacc.Bacc("TRN2", target_bir_lowering=False, debug=False,
                   num_devices=N_CORES)

    sections = []
    for gname, cap, f8 in (("Ab", Qb, False), ("Af", QfA, True),
                           ("Bb", Qb, False), ("Bf", QfB, True)):
        sizes = chunk_list(cap)
        nfull = sum(1 for s in sizes if s == NT)
        tail = cap % NT if cap >= NT else cap
        if tail == cap and cap >= NT:
            tail = 0
        dt = F8 if f8 else BF16
        sd = {
            "name": gname, "sizes": sizes, "tail": tail, "nfull": nfull,
            "f8": f8, "dt": dt,
            "w1": nc.dram_tensor(f"w{gname}1", [128, KC, HD], dt,
                                 kind="ExternalInput"),
            "b1": nc.dram_tensor(f"b{gname}1", [128, KH], F32,
                                 kind="ExternalInput"),
            "w2": nc.dram_tensor(f"w{gname}2", [128, KH, D], dt,
                                 kind="ExternalInput"),
            "b2": nc.dram_tensor(f"b{gname}2", [128, MC], F32,
                                 kind="ExternalInput"),
        }
        if nfull:
            sd["x"] = nc.dram_tensor(f"x{gname}", [nfull, 128, KC, NT], dt,
                                     kind="ExternalInput")
            sd["y"] = nc.dram_tensor(f"y{gname}", [nfull, 128, MC, NT], BF16,
                                     kind="ExternalOutput")
        if tail:
            sd["xt"] = nc.dram_tensor(f"x{gname}t", [128, KC, tail], dt,
                                      kind="ExternalInput")
            sd["yt"] = nc.dram_tensor(f"y{gname}t", [128, MC, tail], BF16,
                                      kind="ExternalOutput")
        sections.append(sd)

    def x_view(sd, n):
        # chunk n in processing order; tail (if any) is chunk 0
        if sd["tail"]:
            return sd["xt"].ap() if n == 0 else sd["x"].ap()[n - 1]
        return sd["x"].ap()[n]

    def y_view(sd, n):
        if sd["tail"]:
            return sd["yt"].ap() if n == 0 else sd["y"].ap()[n - 1]
        return sd["y"].ap()[n]

    # pipeline stages: (section, chunk) in processing order
    stages = [(sd, n) for sd in sections for n in range(len(sd["sizes"]))]
    nst = len(stages)

    with tile.TileContext(nc) as tc:
        with (
            tc.tile_pool(name="weights", bufs=(2 if wdouble else 1)) as wpool,
            tc.tile_pool(name="wf8", bufs=1) as wf8pool,
            tc.tile_pool(name="consts", bufs=(2 if wdouble else 1)) as cpool,
            tc.tile_pool(name="xin", bufs=3) as xpool,
            tc.tile_pool(name="hmid", bufs=4) as hpool,
            tc.tile_pool(name="yout", bufs=4) as ypool,
            tc.tile_pool(name="ph", bufs=3, space="PSUM") as phpool,
            tc.tile_pool(name="py", bufs=5, space="PSUM") as pypool,
        ):
            if hw_loop:
                loop_cm = tc.For_i(0, repeat, 1,
                                   hint_engines=(mybir.EngineType.PE,
                                                 mybir.EngineType.Activation,
                                                 mybir.EngineType.DVE,
                                                 mybir.EngineType.SP),
                                   staggered_reset=True)
            else:
                import contextlib
                loop_cm = contextlib.nullcontext()
                unroll = unroll * repeat
                repeat = 1
            state: dict = {}

            def alloc_tiles():
                for sd in sections:
                    g = sd["name"]
                    wp = wf8pool if sd["f8"] else wpool
                    state[f"w1{g}"] = wp.tile([128, KC, HD], sd["dt"],
                                              tag=f"w1{g}", name=f"w1{g}_sb")
                    state[f"w2{g}"] = wp.tile([128, KH, D], sd["dt"],
                                              tag=f"w2{g}", name=f"w2{g}_sb")
                    state[f"b1{g}"] = cpool.tile([128, KH], F32,
                                                 tag=f"b1{g}",
                                                 name=f"b1{g}_sb")
                    state[f"b2{g}"] = cpool.tile([128, MC], F32,
                                                 tag=f"b2{g}",
                                                 name=f"b2{g}_sb")

            def emit_h_mc(sd, n, mc, x_sb, h_sb):
                sz = sd["sizes"][n]
                g = sd["name"]
                ph = phpool.tile([128, NT], F32, tag="ph", name="ph")
                if sd["f8"]:
                    for k2 in range(KC // 2):
                        nc.tensor.matmul(
                            ph[:, :sz],
                            state[f"w1{g}"][:, 2 * k2:2 * k2 + 2,
                                            bass.ts(mc, 128)],
                            x_sb[:, 2 * k2:2 * k2 + 2, :sz],
                            start=(k2 == 0), stop=(k2 == KC // 2 - 1),
                            perf_mode=DR,
                        )
                else:
                    for kc in range(KC):
                        nc.tensor.matmul(
                            ph[:, :sz],
                            state[f"w1{g}"][:, kc, bass.ts(mc, 128)],
                            x_sb[:, kc, :sz],
                            start=(kc == 0), stop=(kc == KC - 1),
                        )
                # h = relu(ph + b1); fp8 sections split across Act/DVE (their
                # per-PE-time elementwise load is 2x), bf16 stays on DVE
                if sd["f8"] and mc % 2 == 0:
                    nc.scalar.activation(
                        h_sb[:, mc, :sz], ph[:, :sz],
                        mybir.ActivationFunctionType.Relu,
                        bias=state[f"b1{g}"][:, mc:mc + 1],
                    )
                else:
                    nc.vector.tensor_scalar(
                        h_sb[:, mc, :sz], ph[:, :sz],
                        state[f"b1{g}"][:, mc:mc + 1], 0.0,
                        mybir.AluOpType.add, mybir.AluOpType.max,
                    )

            def emit_y_mc(sd, n, mc, h_sb, y_sb):
                sz = sd["sizes"][n]
                g = sd["name"]
                py = pypool.tile([128, NT], F32, tag="py", name="py")
                if sd["f8"]:
                    for k2 in range(KH // 2):
                        nc.tensor.matmul(
                            py[:, :sz],
                            state[f"w2{g}"][:, 2 * k2:2 * k2 + 2,
                                            bass.ts(mc, 128)],
                            h_sb[:, 2 * k2:2 * k2 + 2, :sz],
                            start=(k2 == 0), stop=(k2 == KH // 2 - 1),
                            perf_mode=DR,
                        )
                else:
                    for kh in range(KH):
                        nc.tensor.matmul(
                            py[:, :sz],
                            state[f"w2{g}"][:, kh, bass.ts(mc, 128)],
                            h_sb[:, kh, :sz],
                            start=(kh == 0), stop=(kh == KH - 1),
                        )
                # y = py + b2  (alternate Act/DVE so neither engine's
                # latency tail delays PSUM-bank recycling)
                if mc % 2 == 0:
                    nc.scalar.activation(
                        y_sb[:, mc, :sz], py[:, :sz],
                        mybir.ActivationFunctionType.Identity,
                        bias=state[f"b2{g}"][:, mc:mc + 1],
                    )
                else:
                    nc.vector.tensor_scalar(
                        y_sb[:, mc, :sz], py[:, :sz],
                        state[f"b2{g}"][:, mc:mc + 1], None,
                        mybir.AluOpType.add,
                    )

            def emit_w_dma(sd, which):
                g = sd["name"]
                if which == 1:
                    nc.sync.dma_start(state[f"w1{g}"][:], sd["w1"].ap())
                    nc.sync.dma_start(state[f"b1{g}"][:], sd["b1"].ap())
                else:
                    nc.sync.dma_start(state[f"w2{g}"][:], sd["w2"].ap())
                    nc.sync.dma_start(state[f"b2{g}"][:], sd["b2"].ap())

            def emit_prologue(x_tiles):
                # Interleaved wAb1/x0 DMAs so the first matmuls wait only on
                # their own slices.
                sd0, n0 = stages[0]
                grp = KC // split_w
                for i in range(split_w):
                    ks = slice(i * grp, (i + 1) * grp)
                    nc.sync.dma_start(state["w1Ab"][:, ks, :],
                                      sd0["w1"].ap()[:, ks, :])
                    nc.sync.dma_start(x_tiles[0][:, ks, :sd0["sizes"][n0]],
                                      x_view(sd0, n0)[:, ks, :])
                nc.sync.dma_start(state["b1Ab"][:], sd0["b1"].ap())

            # remaining weight DMAs spread across early stages
            wdma_sched = {0: (0, 2), 1: (1, 1), 2: (1, 2), 3: (2, 1),
                          4: (2, 2), 5: (3, 1), 6: (3, 2)}

            def emit_pipeline(x_tiles):
                h_tiles = {}
                y_tiles = {}
                for si in range(nst + 1):
                    cur = stages[si] if si < nst else None
                    prev = stages[si - 1] if si > 0 else None
                    if si + 1 < nst:  # prefetch next stage's x
                        sdn, nn = stages[si + 1]
                        xt = xpool.tile([128, KC, NT], sdn["dt"],
                                        tag=("xf" if sdn["f8"] else "xb"),
                                        name=f"xs{si + 1}")
                        nc.sync.dma_start(xt[:, :, :sdn["sizes"][nn]],
                                          x_view(sdn, nn))
                        x_tiles[si + 1] = xt
                    if cur is not None:
                        h_tiles[si] = hpool.tile(
                            [128, KH, NT], cur[0]["dt"],
                            tag=("hf" if cur[0]["f8"] else "hb"),
                            name=f"hs{si}")
                    if prev is not None:
                        y_tiles[si - 1] = ypool.tile([128, MC, NT], BF16,
                                                     tag="y", name=f"ys{si}")
                    for mc in range(MC):
                        if cur is not None and mc < KH:
                            emit_h_mc(cur[0], cur[1], mc, x_tiles[si],
                                      h_tiles[si])
                        if prev is not None:
                            emit_y_mc(prev[0], prev[1], mc, h_tiles[si - 1],
                                      y_tiles[si - 1])
                    if prev is not None:
                        # single writeback per chunk: contiguous rows
                        sdp, np_ = prev
                        nc.sync.dma_start(
                            y_view(sdp, np_)[:, :, :],
                            y_tiles[si - 1][:, :, :sdp["sizes"][np_]])
                    if si in wdma_sched:
                        sec_i, which = wdma_sched[si]
                        emit_w_dma(sections[sec_i], which)
                    x_tiles.pop(si - 1, None)
                    h_tiles.pop(si - 2, None)
                    y_tiles.pop(si - 2, None)

            with loop_cm:
                for _ in range(unroll):
                    alloc_tiles()
                    sd0 = stages[0][0]
                    x0 = xpool.tile([128, KC, NT], sd0["dt"],
                                    tag=("xf" if sd0["f8"] else "xb"),
                                    name="x0")
                    x_tiles = {0: x0}
                    emit_prologue(x_tiles)
                    emit_pipeline(x_tiles)

    nc.compile()
    return nc


_NC_CACHE: dict = {}


def _get_kernel(C, repeat: int = 1, **opts) -> bacc.Bacc:
    key = (tuple(C[:3]), repeat, tuple(sorted(opts.items())))
    if key not in _NC_CACHE:
        _NC_CACHE[key] = build_moe_expert_kernel(tuple(C[:3]), repeat, **opts)
    return _NC_CACHE[key]


def _pad(n):
    return max(CGRAIN, ((n + CGRAIN - 1) // CGRAIN) * CGRAIN)


def dispatch(x, W_gate, b_gate, qb: int = QB):
    """Host-side gate + top-2 dispatch with per-expert precision classes.

    Each expert's routed pairs are sorted by |raw score|; the qb
    highest-|score| pairs form the bf16 class, the rest the fp8 class.
    Returns (xf, per-expert dict lists, C) with
    C = (Qb, QfA, QfB, pairs) and pairs = 4 (bigE, smallE) tuples.
    """
    xf = np.ascontiguousarray(np.asarray(x).reshape(-1, D), dtype=np.float32)
    scores = xf @ np.asarray(W_gate, np.float32) + np.asarray(b_gate, np.float32)
    top2 = np.argpartition(scores, N_EXPERTS - TOP_K, axis=1)[:, -TOP_K:]
    counts = []
    ids_b, wts_b, ids_f, wts_f = [], [], [], []
    qb = min(qb, min(int((top2 == e).any(axis=1).sum())
                     for e in range(N_EXPERTS)) // CGRAIN * CGRAIN)
    for e in range(N_EXPERTS):
        tok = np.nonzero((top2 == e).any(axis=1))[0]
        w = scores[tok, e]
        order = np.argsort(np.abs(w), kind="stable")
        cf = max(0, len(tok) - qb)
        fsel = np.zeros(len(tok), bool)
        fsel[order[:cf]] = True
        ids_b.append(tok[~fsel]); wts_b.append(w[~fsel])
        ids_f.append(tok[fsel]); wts_f.append(w[fsel])
        counts.append(len(tok))
    order = list(np.argsort(-np.asarray(counts), kind="stable"))
    pairs = [(int(order[p]), int(order[7 - p])) for p in range(4)]
    QfA = _pad(max(max(len(ids_f[a]) for a, _ in pairs), 1))
    QfB = _pad(max(max(len(ids_f[b]) for _, b in pairs), 1))
    C = (qb, QfA, QfB, tuple(pairs))
    return xf, (ids_b, wts_b, ids_f, wts_f), C


def pack_rows(a):
    """[(kc kp), n] row-major -> [128, nkc, n] partition-major."""
    nkc = a.shape[0] // 128
    return np.ascontiguousarray(a.reshape(nkc, 128, -1).transpose(1, 0, 2))


def _pack_x(xTe, cap, npdt):
    """xT [D, cnt] -> packed chunk blocks (tail chunk first)."""
    Dd, cnt = xTe.shape
    xp = np.zeros((128, KC, cap), npdt)
    xp[:, :, :cnt] = pack_rows(xTe)
    if cap < NT:
        return None, np.ascontiguousarray(xp)
    tail = cap % NT
    nfull = cap // NT
    xb = np.ascontiguousarray(
        xp[:, :, tail:].reshape(128, KC, nfull, NT).transpose(2, 0, 1, 3))
    xt = np.ascontiguousarray(xp[:, :, :tail]) if tail else None
    return xb, xt


def make_in_maps(parts, xf, disp, C):
    """Build per-core input dicts (packed partition-major blocks)."""
    W1, b1, W2, b2 = parts
    ids_b, wts_b, ids_f, wts_f = disp
    Qb, QfA, QfB, pairs = C
    in_maps = []
    for p in range(4):
        for h in range(2):
            hs = slice(h * HD, (h + 1) * HD)
            m = {}
            for base, e in (("A", pairs[p][0]), ("B", pairs[p][1])):
                for cls, cap, ids in ((f"{base}b", Qb, ids_b[e]),
                                      (f"{base}f",
                                       QfA if base == "A" else QfB,
                                       ids_f[e])):
                    f8 = cls.endswith("f")
                    npdt = F8NP if f8 else BF
                    ws = WSCALE if f8 else 1.0
                    xTe = xf[ids].T.astype(npdt)
                    xb, xt = _pack_x(xTe, cap, npdt)
                    if xb is not None:
                        m[f"x{cls}"] = xb
                    if xt is not None:
                        m[f"x{cls}t"] = xt
                    m[f"w{cls}1"] = pack_rows(
                        (np.asarray(W1[e][:, hs], np.float32) * ws
                         ).astype(npdt))
                    m[f"w{cls}2"] = pack_rows(
                        (np.asarray(W2[e][hs, :], np.float32) * ws
                         ).astype(npdt))
                    m[f"b{cls}1"] = np.ascontiguousarray(
                        (np.asarray(b1[e][hs], np.float32) * ws
                         ).reshape(KH, 128).T)
                    b2v = (np.asarray(b2[e], np.float32) * ws * ws if h == 0
                           else np.zeros(D, np.float32))
                    m[f"b{cls}2"] = np.ascontiguousarray(
                        b2v.reshape(MC, 128).T)
            in_maps.append(m)
    return in_maps


def _unpack_y(r, cls, cap):
    """packed y blocks -> yT [D, cap] fp32 (tail chunk first)."""
    if cap < NT:
        return r[f"y{cls}t"].transpose(1, 0, 2).reshape(D, cap).astype(
            np.float32)
    tail = cap % NT
    nfull = cap // NT
    yb = r[f"y{cls}"].transpose(2, 1, 0, 3).reshape(D, nfull * NT)
    if tail:
        yt = r[f"y{cls}t"].transpose(1, 0, 2).reshape(D, tail)
        yb = np.concatenate([yt, yb], axis=1)
    return yb.astype(np.float32)


def kernel(x, W_gate, b_gate, W1, b1, W2, b2):
    xf, disp, C = dispatch(x, W_gate, b_gate)
    ids_b, wts_b, ids_f, wts_f = disp
    Qb, QfA, QfB, pairs = C
    nc = _get_kernel(C)

    in_maps = make_in_maps((W1, b1, W2, b2), xf, disp, C)
    res = run_bass_kernel_spmd(nc, in_maps, core_ids=list(range(N_CORES)))

    out = np.zeros((N_TOKENS, D), np.float32)
    for p in range(4):
        r0, r1 = res.results[2 * p], res.results[2 * p + 1]
        for base, e in (("A", pairs[p][0]), ("B", pairs[p][1])):
            for cls, cap, ids, wts, scl in (
                    (f"{base}b", Qb, ids_b[e], wts_b[e], 1.0),
                    (f"{base}f", QfA if base == "A" else QfB,
                     ids_f[e], wts_f[e], WSCALE * WSCALE)):
                cnt = len(ids)
                if cnt == 0:
                    continue
                yT = _unpack_y(r0, cls, cap) + _unpack_y(r1, cls, cap)
                out[ids] += yT.T[:cnt] * (wts / scl)[:, None]
    return out.reshape(B, T, D)


# revision 5
# speedup vs baseline: 1.0370x; 1.0370x over previous
"""MoE (top-2 of 8 experts, d=1024) — mixed bf16/fp8 hidden-split Bass kernel
for 8 trn2 cores.

Two stacked ideas:

1. Hidden-split expert parallelism (as before): each expert's MLP is split
   in half along the HIDDEN dimension (512 units each); experts sorted by
   routed-token count and paired big-with-small onto core pairs; core 2p
   takes hidden-half 0 of pair p's two experts, core 2p+1 takes half 1.
   Partial y outputs combine on the host along with the top-2 gate weights.

2. Score-weighted mixed precision: the combine weight of a routed
   (token, expert) pair is its RAW gate score, so pairs with small |score|
   contribute proportionally small absolute error to the output. Each
   expert's routed set is split into a bf16 class (the Qb highest-|score|
   pairs) and an fp8 class (the rest). fp8 pairs run both MLP layers in
   fp8e4m3 with MatmulPerfMode.DoubleRow, which processes a 256-deep
   contraction per pass — measured ~2.13x the sustained bf16 row rate on
   this silicon (the PE's sustained-power row rate, ~2.03 Grows/s bf16, is
   the binding roofline; LDWEIGHTS and PSUM bank patterns measure free).
   Device-accurate numpy emulation puts the end-to-end max-rel error at
   ~1.2e-2 for Qb=1024 (50% fp8) vs the 2e-2 gate.

   fp8 scaling: W1, W2 are quantized as fp8(16*W); biases pre-scaled on the
   host (16*b1, 256*b2); h is stored as fp8(16*h_true) straight out of the
   bias+relu op, and the final partial y (= 256*y_true) is stored bf16 and
   divided by 256 during the host combine. No extra device ops vs bf16.

Per-core program (SPMD, identical): four sections Ab(Qb) Af(QfA) Bb(Qb)
Bf(QfB), where Qb is a uniform bf16 capacity (zero bf16 padding) and the
per-expert count variance lives in the half-cost fp8 sections.
Software-pipelined chunks across sections as before: chunk s's layer-1
matmuls interleave with chunk s-1's layer-2 matmuls; tail chunks first;
all DRAM packed 128-partition-major.
"""

import numpy as np
import ml_dtypes

import concourse.bass as bass
import concourse.mybir as mybir
import concourse.tile as tile
from concourse import bacc
from concourse.bass_utils import run_bass_kernel_spmd

# Problem shapes (hardcoded per contract)
D = 1024   # d_model == d_hidden
HD = 512   # hidden half per shard
N_EXPERTS = 8
TOP_K = 2
N_CORES = 8
B, T = 4, 2048
N_TOKENS = B * T

F32 = mybir.dt.float32
BF16 = mybir.dt.bfloat16
F8 = mybir.dt.float8e4
BF = ml_dtypes.bfloat16
F8NP = ml_dtypes.float8_e4m3
KC = D // 128    # layer-1 contraction chunks (8)
KH = HD // 128   # layer-2 contraction chunks (4) == layer-1 output chunks
MC = D // 128    # layer-2 output chunks (8)
NT = 512         # tokens per matmul (moving free dim; one PSUM bank fp32)
CGRAIN = 32      # capacity granularity

QB = 736         # bf16 pairs per expert (rest go fp8); tune vs error budget
WSCALE = 16.0    # fp8 weight scale; y partials come out scaled by WSCALE^2
DR = mybir.MatmulPerfMode.DoubleRow


def chunk_list(C):
    """Chunk sizes in processing order: tail (if any) first, then 512s."""
    assert C % CGRAIN == 0 and C > 0
    if C < NT:
        return [C]
    sizes = [NT] * (C // NT)
    if C % NT:
        sizes = [C % NT] + sizes
    return sizes


def build_moe_expert_kernel(C, repeat: int = 1, split_w: int = 2,
                            wdouble: bool = True, unroll: int = 1,
                            hw_loop: bool = True) -> bacc.Bacc:
    """C = (Qb, QfA, QfB). Sections: Ab(Qb, bf16), Af(QfA, fp8),
    Bb(Qb, bf16), Bf(QfB, fp8).

    DRAM inputs (packed partition-major):
      x{g}  [nfull, 128, KC, NT] (+ x{g}t [128, KC, tail])   bf16 | fp8
      w1{g} [128, KC, HD], w2{g} [128, KH, D]                bf16 | fp8
      b1{g} [128, KH], b2{g} [128, MC]                       f32 (prescaled
                                                             for fp8)
    Outputs: y{g} [nfull, 128, MC, NT] (+ y{g}t) bf16 partial sums
    (fp8 sections' y is 256x the true partial; host divides).
    `repeat` wraps the body in a hardware loop for slope timing; `wdouble`
    double-buffers bf16 weights and unrolls 2 iterations per trip so
    iteration k+1's weight reloads overlap iteration k's compute. fp8
    weights/biases are single-buffered (their reload window is wide).
    """
    Qb, QfA, QfB = C
    if wdouble:
        unroll = 2
        repeat = max(1, (repeat + 1) // 2)
    nc = bacc.Bacc("TRN2", target_bir_lowering=False, debug=False,
                   num_devices=N_CORES)

    sections = []
    for gname, cap, f8 in (("Ab", Qb, False), ("Af", QfA, True),
                           ("Bb", Qb, False), ("Bf", QfB, True)):
        sizes = chunk_list(cap)
        nfull = sum(1 for s in sizes if s == NT)
        tail = cap % NT if cap >= NT else cap
        if tail == cap and cap >= NT:
            tail = 0
        dt = F8 if f8 else BF16
        sd = {
            "name": gname, "sizes": sizes, "tail": tail, "nfull": nfull,
            "f8": f8, "dt": dt,
            "w1": nc.dram_tensor(f"w{gname}1", [128, KC, HD], dt,
                                 kind="ExternalInput"),
            "b1": nc.dram_tensor(f"b{gname}1", [128, KH], F32,
                                 kind="ExternalInput"),
            "w2": nc.dram_tensor(f"w{gname}2", [128, KH, D], dt,
                                 kind="ExternalInput"),
            "b2": nc.dram_tensor(f"b{gname}2", [128, MC], F32,
                                 kind="ExternalInput"),
        }
        if nfull:
            sd["x"] = nc.dram_tensor(f"x{gname}", [nfull, 128, KC, NT], dt,
                                     kind="ExternalInput")
            sd["y"] = nc.dram_tensor(f"y{gname}", [nfull, 128, MC, NT], BF16,
                                     kind="ExternalOutput")
        if tail:
            sd["xt"] = nc.dram_tensor(f"x{gname}t", [128, KC, tail], dt,
                                      kind="ExternalInput")
            sd["yt"] = nc.dram_tensor(f"y{gname}t", [128, MC, tail], BF16,
                                      kind="ExternalOutput")
        sections.append(sd)

    def x_view(sd, n):
        # chunk n in processing order; tail (if any) is chunk 0
        if sd["tail"]:
            return sd["xt"].ap() if n == 0 else sd["x"].ap()[n - 1]
        return sd["x"].ap()[n]

    def y_view(sd, n):
        if sd["tail"]:
            return sd["yt"].ap() if n == 0 else sd["y"].ap()[n - 1]
        return sd["y"].ap()[n]

    # pipeline stages: (section, chunk) in processing order
    stages = [(sd, n) for sd in sections for n in range(len(sd["sizes"]))]
    nst = len(stages)

    with tile.TileContext(nc) as tc:
        with (
            tc.tile_pool(name="weights", bufs=(2 if wdouble else 1)) as wpool,
            tc.tile_pool(name="wf8", bufs=1) as wf8pool,
            tc.tile_pool(name="consts", bufs=(2 if wdouble else 1)) as cpool,
            tc.tile_pool(name="xin", bufs=3) as xpool,
            tc.tile_pool(name="hmid", bufs=4) as hpool,
            tc.tile_pool(name="yout", bufs=4) as ypool,
            tc.tile_pool(name="ph", bufs=3, space="PSUM") as phpool,
            tc.tile_pool(name="py", bufs=5, space="PSUM") as pypool,
        ):
            if hw_loop:
                loop_cm = tc.For_i(0, repeat, 1,
                                   hint_engines=(mybir.EngineType.PE,
                                                 mybir.EngineType.Activation,
                                                 mybir.EngineType.DVE,
                                                 mybir.EngineType.SP),
                                   staggered_reset=True)
            else:
                import contextlib
                loop_cm = contextlib.nullcontext()
                unroll = unroll * repeat
                repeat = 1
            state: dict = {}

            def alloc_tiles():
                for sd in sections:
                    g = sd["name"]
                    wp = wf8pool if sd["f8"] else wpool
                    state[f"w1{g}"] = wp.tile([128, KC, HD], sd["dt"],
                                              tag=f"w1{g}", name=f"w1{g}_sb")
                    state[f"w2{g}"] = wp.tile([128, KH, D], sd["dt"],
                                              tag=f"w2{g}", name=f"w2{g}_sb")
                    state[f"b1{g}"] = cpool.tile([128, KH], F32,
                                                 tag=f"b1{g}",
                                                 name=f"b1{g}_sb")
                    state[f"b2{g}"] = cpool.tile([128, MC], F32,
                                                 tag=f"b2{g}",
                                                 name=f"b2{g}_sb")

            def emit_h_mc(sd, n, mc, x_sb, h_sb):
                sz = sd["sizes"][n]
                g = sd["name"]
                ph = phpool.tile([128, NT], F32, tag="ph", name="ph")
                if sd["f8"]:
                    for k2 in range(KC // 2):
                        nc.tensor.matmul(
                            ph[:, :sz],
                            state[f"w1{g}"][:, 2 * k2:2 * k2 + 2,
                                            bass.ts(mc, 128)],
                            x_sb[:, 2 * k2:2 * k2 + 2, :sz],
                            start=(k2 == 0), stop=(k2 == KC // 2 - 1),
                            perf_mode=DR,
                        )
                else:
                    for kc in range(KC):
                        nc.tensor.matmul(
                            ph[:, :sz],
                            state[f"w1{g}"][:, kc, bass.ts(mc, 128)],
                            x_sb[:, kc, :sz],
                            start=(kc == 0), stop=(kc == KC - 1),
                        )
                # h = relu(ph + b1); fp8 sections split across Act/DVE (their
                # per-PE-time elementwise load is 2x), bf16 stays on DVE
                if sd["f8"] and mc % 2 == 0:
                    nc.scalar.activation(
                        h_sb[:, mc, :sz], ph[:, :sz],
                        mybir.ActivationFunctionType.Relu,
                        bias=state[f"b1{g}"][:, mc:mc + 1],
                    )
                else:
                    nc.vector.tensor_scalar(
                        h_sb[:, mc, :sz], ph[:, :sz],
                        state[f"b1{g}"][:, mc:mc + 1], 0.0,
                        mybir.AluOpType.add, mybir.AluOpType.max,
                    )

            def emit_y_mc(sd, n, mc, h_sb, y_sb):
                sz = sd["sizes"][n]
                g = sd["name"]
                py = pypool.tile([128, NT], F32, tag="py", name="py")
                if sd["f8"]:
                    for k2 in range(KH // 2):
                        nc.tensor.matmul(
                            py[:, :sz],
                            state[f"w2{g}"][:, 2 * k2:2 * k2 + 2,
                                            bass.ts(mc, 128)],
                            h_sb[:, 2 * k2:2 * k2 + 2, :sz],
                            start=(k2 == 0), stop=(k2 == KH // 2 - 1),
                            perf_mode=DR,
                        )
                else:
                    for kh in range(KH):
                        nc.tensor.matmul(
                            py[:, :sz],
                            state[f"w2{g}"][:, kh, bass.ts(mc, 128)],
                            h_sb[:, kh, :sz],
                            start=(kh == 0), stop=(kh == KH - 1),
                        )
                # y = py + b2  (alternate Act/DVE so neither engine's
                # latency tail delays PSUM-bank recycling)
                if mc % 2 == 0:
                    nc.scalar.activation(
                        y_sb[:, mc, :sz], py[:, :sz],
                        mybir.ActivationFunctionType.Identity,
                        bias=state[f"b2{g}"][:, mc:mc + 1],
                    )
                else:
                    nc.vector.tensor_scalar(
                        y_sb[:, mc, :sz], py[:, :sz],
                        state[f"b2{g}"][:, mc:mc + 1], None,
                        mybir.AluOpType.add,
                    )

            def emit_w_dma(sd, which):
                g = sd["name"]
                if which == 1:
                    nc.sync.dma_start(state[f"w1{g}"][:], sd["w1"].ap())
                    nc.sync.dma_start(state[f"b1{g}"][:], sd["b1"].ap())
                else:
                    nc.sync.dma_start(state[f"w2{g}"][:], sd["w2"].ap())
                    nc.sync.dma_start(state[f"b2{g}"][:], sd["b2"].ap())

            def emit_prologue(x_tiles):
                # Interleaved wAb1/x0 DMAs so the first matmuls wait only on
                # their own slices.
                sd0, n0 = stages[0]
                grp = KC // split_w
                for i in range(split_w):
                    ks = slice(i * grp, (i + 1) * grp)
                    nc.sync.dma_start(state["w1Ab"][:, ks, :],
                                      sd0["w1"].ap()[:, ks, :])
                    nc.sync.dma_start(x_tiles[0][:, ks, :sd0["sizes"][n0]],
                                      x_view(sd0, n0)[:, ks, :])
                nc.sync.dma_start(state["b1Ab"][:], sd0["b1"].ap())

            # remaining weight DMAs spread across early stages
            wdma_sched = {0: (0, 2), 1: (1, 1), 2: (1, 2), 3: (2, 1),
                          4: (2, 2), 5: (3, 1), 6: (3, 2)}

            def emit_pipeline(x_tiles):
                h_tiles = {}
                y_tiles = {}
                for si in range(nst + 1):
                    cur = stages[si] if si < nst else None
                    prev = stages[si - 1] if si > 0 else None
                    if si + 1 < nst:  # prefetch next stage's x
                        sdn, nn = stages[si + 1]
                        xt = xpool.tile([128, KC, NT], sdn["dt"],
                                        tag=("xf" if sdn["f8"] else "xb"),
                                        name=f"xs{si + 1}")
                        nc.sync.dma_start(xt[:, :, :sdn["sizes"][nn]],
                                          x_view(sdn, nn))
                        x_tiles[si + 1] = xt
                    if cur is not None:
                        h_tiles[si] = hpool.tile(
                            [128, KH, NT], cur[0]["dt"],
                            tag=("hf" if cur[0]["f8"] else "hb"),
                            name=f"hs{si}")
                    if prev is not None:
                        y_tiles[si - 1] = ypool.tile([128, MC, NT], BF16,
                                                     tag="y", name=f"ys{si}")
                    for mc in range(MC):
                        if cur is not None and mc < KH:
                            emit_h_mc(cur[0], cur[1], mc, x_tiles[si],
                                      h_tiles[si])
                        if prev is not None:
                            emit_y_mc(prev[0], prev[1], mc, h_tiles[si - 1],
                                      y_tiles[si - 1])
                    if prev is not None:
                        # single writeback per chunk: contiguous rows
                        sdp, np_ = prev
                        nc.sync.dma_start(
                            y_view(sdp, np_)[:, :, :],
                            y_tiles[si - 1][:, :, :sdp["sizes"][np_]])
                    if si in wdma_sched:
                        sec_i, which = wdma_sched[si]
                        emit_w_dma(sections[sec_i], which)
                    x_tiles.pop(si - 1, None)
                    h_tiles.pop(si - 2, None)
                    y_tiles.pop(si - 2, None)

            with loop_cm:
                for _ in range(unroll):
                    alloc_tiles()
                    sd0 = stages[0][0]
                    x0 = xpool.tile([128, KC, NT], sd0["dt"],
                                    tag=("xf" if sd0["f8"] else "xb"),
                                    name="x0")
                    x_tiles = {0: x0}
                    emit_prologue(x_tiles)
                    emit_pipeline(x_tiles)

    nc.compile()
    return nc


_NC_CACHE: dict = {}


def _get_kernel(C, repeat: int = 1, **opts) -> bacc.Bacc:
    key = (tuple(C[:3]), repeat, tuple(sorted(opts.items())))
    if key not in _NC_CACHE:
        _NC_CACHE[key] = build_moe_expert_kernel(tuple(C[:3]), repeat, **opts)
    return _NC_CACHE[key]


def _pad(n):
    return max(CGRAIN, ((n + CGRAIN - 1) // CGRAIN) * CGRAIN)


def dispatch(x, W_gate, b_gate, qb: int = QB):
    """Host-side gate + top-2 dispatch with per-expert precision classes.

    Each expert's routed pairs are sorted by |raw score|; the qb
    highest-|score| pairs form the bf16 class, the rest the fp8 class.
    Returns (xf, per-expert dict lists, C) with
    C = (Qb, QfA, QfB, pairs) and pairs = 4 (bigE, smallE) tuples.
    """
    xf = np.ascontiguousarray(np.asarray(x).reshape(-1, D), dtype=np.float32)
    scores = xf @ np.asarray(W_gate, np.float32) + np.asarray(b_gate, np.float32)
    top2 = np.argpartition(scores, N_EXPERTS - TOP_K, axis=1)[:, -TOP_K:]
    counts = []
    ids_b, wts_b, ids_f, wts_f = [], [], [], []
    qb = min(qb, min(int((top2 == e).any(axis=1).sum())
                     for e in range(N_EXPERTS)) // CGRAIN * CGRAIN)
    for e in range(N_EXPERTS):
        tok = np.nonzero((top2 == e).any(axis=1))[0]
        w = scores[tok, e]
        order = np.argsort(np.abs(w), kind="stable")
        cf = max(0, len(tok) - qb)
        fsel = np.zeros(len(tok), bool)
        fsel[order[:cf]] = True
        ids_b.append(tok[~fsel]); wts_b.append(w[~fsel])
        ids_f.append(tok[fsel]); wts_f.append(w[fsel])
        counts.append(len(tok))
    order = list(np.argsort(-np.asarray(counts), kind="stable"))
    pairs = [(int(order[p]), int(order[7 - p])) for p in range(4)]
    QfA = _pad(max(max(len(ids_f[a]) for a, _ in pairs), 1))
    QfB = _pad(max(max(len(ids_f[b]) for _, b in pairs), 1))
    C = (qb, QfA, QfB, tuple(pairs))
    return xf, (ids_b, wts_b, ids_f, wts_f), C


def pack_rows(a):
    """[(kc kp), n] row-major -> [128, nkc, n] partition-major."""
    nkc = a.shape[0] // 128
    return np.ascontiguousarray(a.reshape(nkc, 128, -1).transpose(1, 0, 2))


def _pack_x(xTe, cap, npdt):
    """xT [D, cnt] -> packed chunk blocks (tail chunk first)."""
    Dd, cnt = xTe.shape
    xp = np.zeros((128, KC, cap), npdt)
    xp[:, :, :cnt] = pack_rows(xTe)
    if cap < NT:
        return None, np.ascontiguousarray(xp)
    tail = cap % NT
    nfull = cap // NT
    xb = np.ascontiguousarray(
        xp[:, :, tail:].reshape(128, KC, nfull, NT).transpose(2, 0, 1, 3))
    xt = np.ascontiguousarray(xp[:, :, :tail]) if tail else None
    return xb, xt


def make_in_maps(parts, xf, disp, C):
    """Build per-core input dicts (packed partition-major blocks)."""
    W1, b1, W2, b2 = parts
    ids_b, wts_b, ids_f, wts_f = disp
    Qb, QfA, QfB, pairs = C
    in_maps = []
    for p in range(4):
        for h in range(2):
            hs = slice(h * HD, (h + 1) * HD)
            m = {}
            for base, e in (("A", pairs[p][0]), ("B", pairs[p][1])):
                for cls, cap, ids in ((f"{base}b", Qb, ids_b[e]),
                                      (f"{base}f",
                                       QfA if base == "A" else QfB,
                                       ids_f[e])):
                    f8 = cls.endswith("f")
                    npdt = F8NP if f8 else BF
                    ws = WSCALE if f8 else 1.0
                    xTe = xf[ids].T.astype(npdt)
                    xb, xt = _pack_x(xTe, cap, npdt)
                    if xb is not None:
                        m[f"x{cls}"] = xb
                    if xt is not None:
                        m[f"x{cls}t"] = xt
                    m[f"w{cls}1"] = pack_rows(
                        (np.asarray(W1[e][:, hs], np.float32) * ws
                         ).astype(npdt))
                    m[f"w{cls}2"] = pack_rows(
                        (np.asarray(W2[e][hs, :], np.float32) * ws
                         ).astype(npdt))
                    m[f"b{cls}1"] = np.ascontiguousarray(
                        (np.asarray(b1[e][hs], np.float32) * ws
                         ).reshape(KH, 128).T)
                    b2v = (np.asarray(b2[e], np.float32) * ws * ws if h == 0
                           else np.zeros(D, np.float32))
                    m[f"b{cls}2"] = np.ascontiguousarray(
                        b2v.reshape(MC, 128).T)
            in_maps.append(m)
    return in_maps


def _unpack_y(r, cls, cap):
    """packed y blocks -> yT [D, cap] fp32 (tail chunk first)."""
    if cap < NT:
        return r[f"y{cls}t"].transpose(1, 0, 2).reshape(D, cap).astype(
            np.float32)
    tail = cap % NT
    nfull = cap // NT
    yb = r[f"y{cls}"].transpose(2, 1, 0, 3).reshape(D, nfull * NT)
    if tail:
        yt = r[f"y{cls}t"].transpose(1, 0, 2).reshape(D, tail)
        yb = np.concatenate([yt, yb], axis=1)
    return yb.astype(np.float32)


def kernel(x, W_gate, b_gate, W1, b1, W2, b2):
    xf, disp, C = dispatch(x, W_gate, b_gate)
    ids_b, wts_b, ids_f, wts_f = disp
    Qb, QfA, QfB, pairs = C
    nc = _get_kernel(C)

    in_maps = make_in_maps((W1, b1, W2, b2), xf, disp, C)
    res = run_bass_kernel_spmd(nc, in_maps, core_ids=list(range(N_CORES)))

    out = np.zeros((N_TOKENS, D), np.float32)
    for p in range(4):
        r0, r1 = res.results[2 * p], res.results[2 * p + 1]
        for base, e in (("A", pairs[p][0]), ("B", pairs[p][1])):
            for cls, cap, ids, wts, scl in (
                    (f"{base}b", Qb, ids_b[e], wts_b[e], 1.0),
                    (f"{base}f", QfA if base == "A" else QfB,
                     ids_f[e], wts_f[e], WSCALE * WSCALE)):
                cnt = len(ids)
                if cnt == 0:
                    continue
                yT = _unpack_y(r0, cls, cap) + _unpack_y(r1, cls, cap)
                out[ids] += yT.T[:cnt] * (wts / scl)[:, None]
    return out.reshape(B, T, D)
